# revision 5
# baseline (speedup 1.0000x reference)
"""Trainium2 Bass kernel for the GRU decoder (nn_Decoder_13168369730058).

Math (from the reference):
  h0 = encoder_outputs[0, :, -1, :]                   # (128, 512)
  step 1:   h1 = gru_cell(x=0, h0)
  step t>1: h_t = gru_cell(h_{t-1}, h_{t-1})          # carry is (h_new, h_new)
Because x == h from step 2 on, the two GRU matmuls fuse into one with
combined weights Wc = [Wr; -Wz; Whn; Win] (z block NEGATED):
  r   = sigmoid(g_r + b_r)
  omz = sigmoid(g_zn - b_z) = 1 - z
  n   = tanh(g_in + b_in + r * (g_hn + b_hn))
  h'  = h + omz * (n - h)
Step 0 is the same with Wc -> W_hh blocks and no in-matmuls.

On-chip layout is fully transposed (h-dims on partitions, batch on the
free dim); each core runs 16 batch rows.  The step loop runs as a
HARDWARE loop (tc.For_i): a 32-step body executes T/32-1 times after a
32-step unrolled prologue (which covers the special t=0 step).  A fully
unrolled program is instruction-fetch bound (~9us/step at 107k
instructions); the looped body sits in instruction memory and runs at
~5.9us/step.

Per step: 3 bias-seed matmuls + 64 weight matmuls (k-phase-major order),
then a 2-ACT/5-DVE tail using  h' = h + (1-z) * (n - h),  with the z-gate
weights negated so  1-z = sigmoid(psum)  directly.  h is carried fp16 in a
16-slot history tile; one DMA per body writes the block to DRAM at a
loop-variable offset; the host converts fp16 -> fp32.

Distribution: data-parallel over batch, 16 rows per core on 8 cores,
weights replicated; the out_len recurrence is local to each core.
"""

import os
import numpy as np

import concourse.bacc as bacc
import concourse.mybir as mybir
import concourse.tile as tile
from concourse.bass import ds
from concourse.bass_utils import run_bass_kernel_spmd

H = 512
BATCH = 128
N_CORES = int(os.environ.get("GRU_N_CORES", "8"))
T_STEPS = int(os.environ.get("GRU_T_STEPS", "1024"))
B_LOC = BATCH // N_CORES  # local batch per core (16)
KT = H // 128              # 4 k-tiles
S_BLK = 32                 # steps per body / output DMA block

F32 = mybir.dt.float32
F16 = mybir.dt.float16

G_R, G_Z, G_HN, G_IN = 0, 1, 2, 3


def _build(T: int, b: int, S: int = S_BLK):
    assert T % S == 0 and T >= 2 * S
    nblocks = T // S
    C = KT * b  # 64 cols per step slot
    nc = bacc.Bacc()

    wc_d = nc.dram_tensor("wc", [128, 64 * 128], F16, kind="ExternalInput")
    w1_d = nc.dram_tensor("w1", [128, 48 * 128], F16, kind="ExternalInput")
    bst_d = nc.dram_tensor("bst", [16, 128], F16, kind="ExternalInput")
    ones8_d = nc.dram_tensor("ones8", [8, 8 * b], F16, kind="ExternalInput")
    ones4_d = nc.dram_tensor("ones4", [4, 4 * b], F16, kind="ExternalInput")
    h0_d = nc.dram_tensor("h0t", [128, C], F16, kind="ExternalInput")
    out_d = nc.dram_tensor("outT", [nblocks * 128, S * C], F16,
                           kind="ExternalOutput")

    sig = mybir.ActivationFunctionType.Sigmoid
    tanh = mybir.ActivationFunctionType.Tanh
    ident = mybir.ActivationFunctionType.Identity

    with tile.TileContext(nc) as tc:
        with (
            tc.tile_pool(name="singles", bufs=1) as singles,
            tc.tile_pool(name="psum", bufs=1, space="PSUM") as psum,
        ):
            w_sb = singles.tile([128, 64 * 128], F16)
            nc.sync.dma_start(w_sb[:], wc_d[:])
            w1_sb = singles.tile([128, 48 * 128], F16)
            nc.sync.dma_start(w1_sb[:], w1_d[:])
            brz_sb = singles.tile([8, 128], F16)
            nc.sync.dma_start(brz_sb[:], bst_d[0:8])
            bhn_sb = singles.tile([4, 128], F16)
            nc.sync.dma_start(bhn_sb[:], bst_d[8:12])
            bin_sb = singles.tile([4, 128], F16)
            nc.sync.dma_start(bin_sb[:], bst_d[12:16])
            ones8_sb = singles.tile([8, 8 * b], F16)
            nc.sync.dma_start(ones8_sb[:], ones8_d[:])
            ones4_sb = singles.tile([4, 4 * b], F16)
            nc.sync.dma_start(ones4_sb[:], ones4_d[:])
            h0_sb = singles.tile([128, C], F16)
            nc.sync.dma_start(h0_sb[:], h0_d[:])
            hist = singles.tile([128, S * C], F16)

            # 2 sets of psum/work tiles, even/odd steps
            rz_ps, hn_ps, in_ps = [], [], []
            sig_t, rhn_t, pre_t, n_tt, nmh_t, homz_t = [], [], [], [], [], []
            for st in range(2):
                rz_ps.append(psum.tile([128, 8 * b], F32, name=f"rz{st}",
                                       tag=f"rz{st}"))
                hn_ps.append(psum.tile([128, 4 * b], F32, name=f"hn{st}",
                                       tag=f"hn{st}"))
                in_ps.append(psum.tile([128, 4 * b], F32, name=f"in{st}",
                                       tag=f"in{st}"))
                sig_t.append(singles.tile([128, 8 * b], F32, name=f"sg{st}"))
                rhn_t.append(singles.tile([128, 4 * b], F32, name=f"rh{st}"))
                pre_t.append(singles.tile([128, 4 * b], F32, name=f"pr{st}"))
                n_tt.append(singles.tile([128, 4 * b], F32, name=f"nn{st}"))
                nmh_t.append(singles.tile([128, 4 * b], F32, name=f"nm{st}"))
                homz_t.append(singles.tile([128, 4 * b], F32, name=f"hz{st}"))

            # Warm-up: each engine observes the init DMAs once.
            nc.tensor.matmul(hn_ps[0][:, 0:8], w_sb[:, 0:128], w_sb[:, 0:8],
                             start=True, stop=True, skip_group_check=True)
            nc.tensor.matmul(hn_ps[0][:, 0:8], w1_sb[:, 0:128],
                             w1_sb[:, 0:8], start=True, stop=True,
                             skip_group_check=True)
            nc.tensor.matmul(hn_ps[0][:, 0:b], brz_sb[:], ones8_sb[:, 0:b],
                             start=True, stop=True, skip_group_check=True)
            nc.tensor.matmul(hn_ps[0][:, 0:b], bhn_sb[:], ones4_sb[:, 0:b],
                             start=True, stop=True, skip_group_check=True)
            nc.tensor.matmul(hn_ps[0][:, 0:b], bin_sb[:], ones4_sb[:, 0:b],
                             start=True, stop=True, skip_group_check=True)
            nc.tensor.matmul(hn_ps[0][:, 0:8], w_sb[:, 0:128], h0_sb[:, 0:8],
                             start=True, stop=True, skip_group_check=True)
            warm_a = singles.tile([128, 1], F32, name="warm_a")
            nc.scalar.activation(warm_a[:], h0_sb[:, 0:1], ident)
            warm_d = singles.tile([128, 8], F32, name="warm_d")
            nc.vector.tensor_copy(warm_d[:], h0_sb[:, 0:8])

            def gru_step(j, first, prev_ap):
                st = j % 2
                rz, hn, inp = rz_ps[st], hn_ps[st], in_ps[st]
                w = w1_sb if first else w_sb

                nc.tensor.matmul(rz[:], brz_sb[:], ones8_sb[:],
                                 start=True, stop=False, skip_group_check=True)
                nc.tensor.matmul(hn[:], bhn_sb[:], ones4_sb[:],
                                 start=True, stop=False, skip_group_check=True)
                nc.tensor.matmul(inp[:], bin_sb[:], ones4_sb[:],
                                 start=True, stop=first, skip_group_check=True)

                def mm(g, tt, k, stop):
                    if g == G_HN:
                        ps, col = hn, tt * b
                    elif g == G_IN:
                        ps, col = inp, tt * b
                    else:
                        ps, col = rz, (tt if g == G_R else 4 + tt) * b
                    blk = ((g * 4 + tt) * 4 + k) * 128
                    nc.tensor.matmul(
                        ps[:, col : col + b],
                        w[:, blk : blk + 128],
                        prev_ap[:, k * b : (k + 1) * b],
                        start=False, stop=stop, skip_group_check=True)

                gates = (G_R, G_Z, G_HN) if first else (G_R, G_Z, G_HN, G_IN)
                for k in (0, 1, 2):
                    for tt in range(KT):
                        for g in gates:
                            mm(g, tt, k, False)
                for g in gates:
                    for tt in range(KT):
                        mm(g, tt, 3, True)

                # tail: h' = h + omz*(n - h);  omz = sigmoid(zn psum)
                nc.scalar.activation(sig_t[st][:], rz[:], sig)
                rT = sig_t[st][:, 0 : 4 * b]
                omzT = sig_t[st][:, 4 * b : 8 * b]
                nc.vector.tensor_mul(rhn_t[st][:], rT, hn[:])
                nc.vector.tensor_add(pre_t[st][:], inp[:], rhn_t[st][:])
                nc.scalar.activation(n_tt[st][:], pre_t[st][:], tanh)
                nc.vector.tensor_sub(nmh_t[st][:], n_tt[st][:], prev_ap)
                nc.vector.tensor_mul(homz_t[st][:], omzT, nmh_t[st][:])
                out_slot = hist[:, j * C : (j + 1) * C]
                nc.vector.tensor_add(out_slot, prev_ap, homz_t[st][:])
                return out_slot

            # prologue: t = 0..S-1 (t=0 special)
            prev = h0_sb[:]
            for j in range(S):
                prev = gru_step(j, j == 0, prev)
            nc.sync.dma_start(out_d[0:128], hist[:])

            # steady-state: 16 steps per iteration, row offset = loop var
            with tc.For_i(128, nblocks * 128, 128) as row:
                for j in range(S):
                    prev = gru_step(j, False, prev)
                nc.sync.dma_start(out_d[ds(row, 128)], hist[:])

    if not nc.is_finalized():
        nc.finalize()
    return nc


def _prep_host(encoder_outputs, W_ih, W_hh, b_ih, b_hh, T, n_cores, b):
    """Shard + lay out host inputs; returns per-core in_maps."""
    W_ih = np.asarray(W_ih, dtype=np.float32)
    W_hh = np.asarray(W_hh, dtype=np.float32)
    b_ih = np.asarray(b_ih, dtype=np.float32)
    b_hh = np.asarray(b_hh, dtype=np.float32)
    enc = np.asarray(encoder_outputs, dtype=np.float32)

    # combined gate weights; z NEGATED so 1-z = sigmoid(psum) directly
    Wg = [
        W_ih[:H] + W_hh[:H],                    # r
        -(W_ih[H : 2 * H] + W_hh[H : 2 * H]),   # -z
        W_hh[2 * H :],                          # hn
        W_ih[2 * H :],                          # in
    ]
    W1g = [W_hh[:H], -W_hh[H : 2 * H], W_hh[2 * H :]]  # step 0

    def blocks_of(gs):
        cols = []
        for Wm in gs:
            WmT = Wm.T  # (512 h-dims, 512 gate rows)
            for tt in range(KT):
                for k in range(KT):
                    cols.append(WmT[128 * k : 128 * (k + 1),
                                    128 * tt : 128 * (tt + 1)])
        return np.ascontiguousarray(
            np.concatenate(cols, axis=1)).astype(np.float16)

    wc_host = blocks_of(Wg)    # (128, 64*128)
    w1_host = blocks_of(W1g)   # (128, 48*128)

    b_r = (b_ih[:H] + b_hh[:H]).reshape(KT, 128)
    b_z = (b_ih[H : 2 * H] + b_hh[H : 2 * H]).reshape(KT, 128)
    b_hn = b_hh[2 * H :].reshape(KT, 128)
    b_in = b_ih[2 * H :].reshape(KT, 128)
    bst = np.ascontiguousarray(
        np.concatenate([b_r, -b_z, b_hn, b_in], axis=0)
    ).astype(np.float16)  # (16, 128)
    ones8 = np.kron(np.eye(8, dtype=np.float16), np.ones((1, b), np.float16))
    ones4 = np.kron(np.eye(4, dtype=np.float16), np.ones((1, b), np.float16))

    h0 = enc[0, :, -1, :]  # (128, 512)
    in_maps = []
    for c in range(n_cores):
        h0c = h0[c * b : (c + 1) * b]  # (b, 512)
        h0t = np.ascontiguousarray(
            h0c.reshape(b, KT, 128).transpose(2, 1, 0).reshape(128, KT * b)
        ).astype(np.float16)
        in_maps.append({
            "wc": wc_host, "w1": w1_host, "bst": bst,
            "ones8": ones8, "ones4": ones4, "h0t": h0t,
        })
    return in_maps


def _gather(results, T, n_cores, b, S=S_BLK):
    out = np.empty((T, BATCH, H), dtype=np.float32)
    for c in range(n_cores):
        oc = results[c]["outT"]  # (T//S*128, S*KT*b) fp16
        out[:, c * b : (c + 1) * b, :] = (
            oc.reshape(T // S, 128, S, KT, b)
            .transpose(0, 2, 4, 3, 1)
            .reshape(T, b, H)
            .astype(np.float32)
        )
    return out


_CACHE = {}
LAST_RESULT = None  # BassKernelResults of the most recent run (for test.py)


def kernel(encoder_outputs, W_ih, W_hh, b_ih, b_hh, out_len):
    global LAST_RESULT
    T = int(out_len)
    assert T == T_STEPS, f"built for T={T_STEPS}, got {T}"
    key = (T, N_CORES)
    if key not in _CACHE:
        _CACHE[key] = _build(T, B_LOC)
    nc = _CACHE[key]

    in_maps = _prep_host(encoder_outputs, W_ih, W_hh, b_ih, b_hh,
                         T, N_CORES, B_LOC)
    res = run_bass_kernel_spmd(nc, in_maps, core_ids=list(range(N_CORES)))
    LAST_RESULT = res
    out = _gather(res.results, T, N_CORES, B_LOC)
    return out.reshape(T * BATCH, 1, H)


# revision 6
# speedup vs baseline: 15.0091x; 15.0091x over previous
"""Trainium2 Bass kernel for the GRU decoder (nn_Decoder_13168369730058).

Math (from the reference):
  h0 = encoder_outputs[0, :, -1, :]                   # (128, 512)
  step 1:   h1 = gru_cell(x=0, h0)
  step t>1: h_t = gru_cell(h_{t-1}, h_{t-1})          # carry is (h_new, h_new)
Because x == h from step 2 on, the two GRU matmuls fuse into one with
combined weights Wc = [Wr; -Wz; Whn; Win] (z block NEGATED):
  r   = sigmoid(g_r + b_r)
  omz = sigmoid(g_zn - b_z) = 1 - z
  n   = tanh(g_in + b_in + r * (g_hn + b_hn))
  h'  = h + omz * (n - h)
Step 0 is the same with Wc -> W_hh blocks and no in-matmuls.

On-chip layout is fully transposed (h-dims on partitions, batch on the
free dim); each core runs 16 batch rows.  The step loop runs as a
HARDWARE loop (tc.For_i): a 32-step body executes T/32-1 times after a
32-step unrolled prologue (which covers the special t=0 step).  A fully
unrolled program is instruction-fetch bound (~9us/step at 107k
instructions); the looped body sits in instruction memory and runs at
~4.9us/step.

Per step: 3 bias-seed matmuls + 64 weight matmuls (k-phase-major order),
then a 2-ACT/5-DVE tail using  h' = h + (1-z) * (n - h),  with the z-gate
weights negated so  1-z = sigmoid(psum)  directly.  h is carried fp16 in a
32-slot history tile; one DMA per body writes the block to DRAM at a
loop-variable offset; the host converts fp16 -> fp32.

Distribution: data-parallel over batch, 16 rows per core on 8 cores,
weights replicated; the out_len recurrence is local to each core.
"""

import os
import numpy as np

import concourse.bacc as bacc
import concourse.mybir as mybir
import concourse.tile as tile
from concourse.bass import ds
from concourse.bass_utils import run_bass_kernel_spmd

H = 512
BATCH = 128
N_CORES = int(os.environ.get("GRU_N_CORES", "8"))
T_STEPS = int(os.environ.get("GRU_T_STEPS", "1024"))
B_LOC = BATCH // N_CORES  # local batch per core (16)
KT = H // 128              # 4 k-tiles
S_BLK = 32                 # steps per body / output DMA block

F32 = mybir.dt.float32
F16 = mybir.dt.float16

G_R, G_Z, G_HN, G_IN = 0, 1, 2, 3


def _build(T: int, b: int, S: int = S_BLK):
    assert T % S == 0 and T >= 2 * S
    nblocks = T // S
    C = KT * b  # 64 cols per step slot
    nc = bacc.Bacc()

    wc_d = nc.dram_tensor("wc", [128, 64 * 128], F16, kind="ExternalInput")
    w1_d = nc.dram_tensor("w1", [128, 48 * 128], F16, kind="ExternalInput")
    bst_d = nc.dram_tensor("bst", [16, 128], F16, kind="ExternalInput")
    ones8_d = nc.dram_tensor("ones8", [8, 8 * b], F16, kind="ExternalInput")
    ones4_d = nc.dram_tensor("ones4", [4, 4 * b], F16, kind="ExternalInput")
    h0_d = nc.dram_tensor("h0t", [128, C], F16, kind="ExternalInput")
    out_d = nc.dram_tensor("outT", [nblocks * 128, S * C], F16,
                           kind="ExternalOutput")

    sig = mybir.ActivationFunctionType.Sigmoid
    tanh = mybir.ActivationFunctionType.Tanh
    ident = mybir.ActivationFunctionType.Identity

    with tile.TileContext(nc) as tc:
        with (
            tc.tile_pool(name="singles", bufs=1) as singles,
            tc.tile_pool(name="psum", bufs=1, space="PSUM") as psum,
        ):
            w_sb = singles.tile([128, 64 * 128], F16)
            nc.sync.dma_start(w_sb[:], wc_d[:])
            w1_sb = singles.tile([128, 48 * 128], F16)
            nc.sync.dma_start(w1_sb[:], w1_d[:])
            brz_sb = singles.tile([8, 128], F16)
            nc.sync.dma_start(brz_sb[:], bst_d[0:8])
            bhn_sb = singles.tile([4, 128], F16)
            nc.sync.dma_start(bhn_sb[:], bst_d[8:12])
            bin_sb = singles.tile([4, 128], F16)
            nc.sync.dma_start(bin_sb[:], bst_d[12:16])
            ones8_sb = singles.tile([8, 8 * b], F16)
            nc.sync.dma_start(ones8_sb[:], ones8_d[:])
            ones4_sb = singles.tile([4, 4 * b], F16)
            nc.sync.dma_start(ones4_sb[:], ones4_d[:])
            h0_sb = singles.tile([128, C], F16)
            nc.sync.dma_start(h0_sb[:], h0_d[:])
            hist = singles.tile([128, S * C], F16)

            # 2 sets of psum/work tiles, even/odd steps
            rz_ps, hn_ps, in_ps = [], [], []
            sig_t, rhn_t, pre_t, n_tt, nmh_t, homz_t = [], [], [], [], [], []
            for st in range(2):
                rz_ps.append(psum.tile([128, 8 * b], F32, name=f"rz{st}",
                                       tag=f"rz{st}"))
                hn_ps.append(psum.tile([128, 4 * b], F32, name=f"hn{st}",
                                       tag=f"hn{st}"))
                in_ps.append(psum.tile([128, 4 * b], F32, name=f"in{st}",
                                       tag=f"in{st}"))
                sig_t.append(singles.tile([128, 8 * b], F32, name=f"sg{st}"))
                rhn_t.append(singles.tile([128, 4 * b], F32, name=f"rh{st}"))
                pre_t.append(singles.tile([128, 4 * b], F32, name=f"pr{st}"))
                n_tt.append(singles.tile([128, 4 * b], F32, name=f"nn{st}"))
                nmh_t.append(singles.tile([128, 4 * b], F32, name=f"nm{st}"))
                homz_t.append(singles.tile([128, 4 * b], F32, name=f"hz{st}"))

            # Warm-up: each engine observes the init DMAs once.
            nc.tensor.matmul(hn_ps[0][:, 0:8], w_sb[:, 0:128], w_sb[:, 0:8],
                             start=True, stop=True, skip_group_check=True)
            nc.tensor.matmul(hn_ps[0][:, 0:8], w1_sb[:, 0:128],
                             w1_sb[:, 0:8], start=True, stop=True,
                             skip_group_check=True)
            nc.tensor.matmul(hn_ps[0][:, 0:b], brz_sb[:], ones8_sb[:, 0:b],
                             start=True, stop=True, skip_group_check=True)
            nc.tensor.matmul(hn_ps[0][:, 0:b], bhn_sb[:], ones4_sb[:, 0:b],
                             start=True, stop=True, skip_group_check=True)
            nc.tensor.matmul(hn_ps[0][:, 0:b], bin_sb[:], ones4_sb[:, 0:b],
                             start=True, stop=True, skip_group_check=True)
            nc.tensor.matmul(hn_ps[0][:, 0:8], w_sb[:, 0:128], h0_sb[:, 0:8],
                             start=True, stop=True, skip_group_check=True)
            warm_a = singles.tile([128, 1], F32, name="warm_a")
            nc.scalar.activation(warm_a[:], h0_sb[:, 0:1], ident)
            warm_d = singles.tile([128, 8], F32, name="warm_d")
            nc.vector.tensor_copy(warm_d[:], h0_sb[:, 0:8])

            def gru_step(j, first, prev_ap):
                st = j % 2
                rz, hn, inp = rz_ps[st], hn_ps[st], in_ps[st]
                w = w1_sb if first else w_sb

                nc.tensor.matmul(rz[:], brz_sb[:], ones8_sb[:],
                                 start=True, stop=False, skip_group_check=True)
                nc.tensor.matmul(hn[:], bhn_sb[:], ones4_sb[:],
                                 start=True, stop=False, skip_group_check=True)
                nc.tensor.matmul(inp[:], bin_sb[:], ones4_sb[:],
                                 start=True, stop=first, skip_group_check=True)

                def mm(g, tt, k, stop):
                    if g == G_HN:
                        ps, col = hn, tt * b
                    elif g == G_IN:
                        ps, col = inp, tt * b
                    else:
                        ps, col = rz, (tt if g == G_R else 4 + tt) * b
                    blk = ((g * 4 + tt) * 4 + k) * 128
                    nc.tensor.matmul(
                        ps[:, col : col + b],
                        w[:, blk : blk + 128],
                        prev_ap[:, k * b : (k + 1) * b],
                        start=False, stop=stop, skip_group_check=True)

                gates = (G_R, G_Z, G_HN) if first else (G_R, G_Z, G_HN, G_IN)
                for k in (0, 1, 2):
                    for tt in range(KT):
                        for g in gates:
                            mm(g, tt, k, False)
                for g in gates:
                    for tt in range(KT):
                        mm(g, tt, 3, True)

                # tail: h' = h + omz*(n - h);  omz = sigmoid(zn psum)
                nc.scalar.activation(sig_t[st][:], rz[:], sig)
                rT = sig_t[st][:, 0 : 4 * b]
                omzT = sig_t[st][:, 4 * b : 8 * b]
                nc.vector.tensor_mul(rhn_t[st][:], rT, hn[:])
                nc.vector.tensor_add(pre_t[st][:], inp[:], rhn_t[st][:])
                nc.scalar.activation(n_tt[st][:], pre_t[st][:], tanh)
                nc.vector.tensor_sub(nmh_t[st][:], n_tt[st][:], prev_ap)
                nc.vector.tensor_mul(homz_t[st][:], omzT, nmh_t[st][:])
                out_slot = hist[:, j * C : (j + 1) * C]
                nc.vector.tensor_add(out_slot, prev_ap, homz_t[st][:])
                return out_slot

            # prologue: t = 0..S-1 (t=0 special)
            prev = h0_sb[:]
            for j in range(S):
                prev = gru_step(j, j == 0, prev)
            nc.sync.dma_start(out_d[0:128], hist[:])

            # steady-state: 16 steps per iteration, row offset = loop var
            with tc.For_i(128, nblocks * 128, 128) as row:
                for j in range(S):
                    prev = gru_step(j, False, prev)
                nc.sync.dma_start(out_d[ds(row, 128)], hist[:])

    if not nc.is_finalized():
        nc.finalize()
    return nc


def _prep_host(encoder_outputs, W_ih, W_hh, b_ih, b_hh, T, n_cores, b):
    """Shard + lay out host inputs; returns per-core in_maps."""
    W_ih = np.asarray(W_ih, dtype=np.float32)
    W_hh = np.asarray(W_hh, dtype=np.float32)
    b_ih = np.asarray(b_ih, dtype=np.float32)
    b_hh = np.asarray(b_hh, dtype=np.float32)
    enc = np.asarray(encoder_outputs, dtype=np.float32)

    # combined gate weights; z NEGATED so 1-z = sigmoid(psum) directly
    Wg = [
        W_ih[:H] + W_hh[:H],                    # r
        -(W_ih[H : 2 * H] + W_hh[H : 2 * H]),   # -z
        W_hh[2 * H :],                          # hn
        W_ih[2 * H :],                          # in
    ]
    W1g = [W_hh[:H], -W_hh[H : 2 * H], W_hh[2 * H :]]  # step 0

    def blocks_of(gs):
        cols = []
        for Wm in gs:
            WmT = Wm.T  # (512 h-dims, 512 gate rows)
            for tt in range(KT):
                for k in range(KT):
                    cols.append(WmT[128 * k : 128 * (k + 1),
                                    128 * tt : 128 * (tt + 1)])
        return np.ascontiguousarray(
            np.concatenate(cols, axis=1)).astype(np.float16)

    wc_host = blocks_of(Wg)    # (128, 64*128)
    w1_host = blocks_of(W1g)   # (128, 48*128)

    b_r = (b_ih[:H] + b_hh[:H]).reshape(KT, 128)
    b_z = (b_ih[H : 2 * H] + b_hh[H : 2 * H]).reshape(KT, 128)
    b_hn = b_hh[2 * H :].reshape(KT, 128)
    b_in = b_ih[2 * H :].reshape(KT, 128)
    bst = np.ascontiguousarray(
        np.concatenate([b_r, -b_z, b_hn, b_in], axis=0)
    ).astype(np.float16)  # (16, 128)
    ones8 = np.kron(np.eye(8, dtype=np.float16), np.ones((1, b), np.float16))
    ones4 = np.kron(np.eye(4, dtype=np.float16), np.ones((1, b), np.float16))

    h0 = enc[0, :, -1, :]  # (128, 512)
    in_maps = []
    for c in range(n_cores):
        h0c = h0[c * b : (c + 1) * b]  # (b, 512)
        h0t = np.ascontiguousarray(
            h0c.reshape(b, KT, 128).transpose(2, 1, 0).reshape(128, KT * b)
        ).astype(np.float16)
        in_maps.append({
            "wc": wc_host, "w1": w1_host, "bst": bst,
            "ones8": ones8, "ones4": ones4, "h0t": h0t,
        })
    return in_maps


def _gather(results, T, n_cores, b, S=S_BLK):
    out = np.empty((T, BATCH, H), dtype=np.float32)
    for c in range(n_cores):
        oc = results[c]["outT"]  # (T//S*128, S*KT*b) fp16
        out[:, c * b : (c + 1) * b, :] = (
            oc.reshape(T // S, 128, S, KT, b)
            .transpose(0, 2, 4, 3, 1)
            .reshape(T, b, H)
            .astype(np.float32)
        )
    return out


_CACHE = {}
LAST_RESULT = None  # BassKernelResults of the most recent run (for test.py)


def kernel(encoder_outputs, W_ih, W_hh, b_ih, b_hh, out_len):
    global LAST_RESULT
    T = int(out_len)
    assert T == T_STEPS, f"built for T={T_STEPS}, got {T}"
    key = (T, N_CORES)
    if key not in _CACHE:
        _CACHE[key] = _build(T, B_LOC)
    nc = _CACHE[key]

    in_maps = _prep_host(encoder_outputs, W_ih, W_hh, b_ih, b_hh,
                         T, N_CORES, B_LOC)
    res = run_bass_kernel_spmd(nc, in_maps, core_ids=list(range(N_CORES)))
    LAST_RESULT = res
    out = _gather(res.results, T, N_CORES, B_LOC)
    return out.reshape(T * BATCH, 1, H)
